# revision 28
# baseline (speedup 1.0000x reference)
"""Causal single-head attention (Q==K) on 8 TRN2 NeuronCores — v3.

v3 restructure: queries split into 512-row groups (8 per batch); each
core of a batch pair owns 4 groups chosen so causal work balances with
near-zero overcompute (8 of 512 key-block-slots wasted per core vs 16
in v2).  Four sequential accumulation phases (one PSUM bank each,
double buffered), processed lightest-first so the panel DMA/projection
pipeline ramps.  Key-block validity for possibly-wasted fixed stream
slots is per-core data (wmask column multiply on the exp'd weights).
"""
import numpy as np
import ml_dtypes

import concourse.bass as bass
import concourse.mybir as mybir
from concourse import bacc, tile
from concourse.bass_utils import run_bass_kernel_spmd

F32 = mybir.dt.float32
BF16 = mybir.dt.bfloat16
EXP = mybir.ActivationFunctionType.Exp

B, T, C, H = 4, 4096, 1024, 64
NCHI = C // 128
NPAN = 8
PAN = 512
GRP = 512                      # query group rows
# structural phase i: query cols [512i, 512i+512), diag kbs 4i..4i+3
LOFF = {
    0: list(range(4, 20)) + list(range(24, 32)) + list(range(20, 24)),
    1: list(range(8, 20)) + list(range(24, 32)),
    2: list(range(12, 20)) + list(range(24, 28)),
    3: list(range(16, 20)),
}
WM = {0: (20, 21, 22, 23), 1: (28, 29, 30, 31),
      2: (24, 25, 26, 27), 3: (16, 17, 18, 19)}
PH_ORDER = [3, 2, 1, 0]        # execution order: lightest first
# role -> structural phase i holds this global 512-group; panels 4..7 hold
# the remaining groups in this order
GA = {0: [7, 4, 3, 0, 1, 5, 2, 6], 1: [6, 5, 2, 1, 0, 7, 3, 4]}
# role -> wmask value per phase (1 = the WM slots are valid keys)
WMVAL = {0: (1.0, 0.0, 1.0, 0.0), 1: (0.0, 1.0, 0.0, 1.0)}


def build_nc():
    nc = bacc.Bacc("TRN2", target_bir_lowering=False, debug=False, num_devices=8)

    xt_d = nc.declare_dram_parameter("xt", [128, NPAN, NCHI, PAN], BF16, isOutput=False)
    wkv_d = nc.declare_dram_parameter("wkv", [128, NCHI, 128], BF16, isOutput=False)
    mk_d = nc.declare_dram_parameter("mk", [128, 128], BF16, isOutput=False)
    wm_d = nc.declare_dram_parameter("wm", [128, 16], F32, isOutput=False)
    eyb_d = nc.declare_dram_parameter("eyb", [64, 64], F32, isOutput=False)
    eyf_d = nc.declare_dram_parameter("eyf", [65, 65], F32, isOutput=False)
    out_d = nc.declare_dram_parameter("out", [4 * GRP, H], F32, isOutput=True)
    out_v = out_d.ap().rearrange("(i p) h -> p i h", p=128)  # [128, 16, 64]

    with tile.TileContext(nc) as tc:
        with (
            tc.tile_pool(name="const", bufs=1) as const,
            tc.tile_pool(name="xt", bufs=12) as xtp,
            tc.tile_pool(name="vh", bufs=2) as vhp,
            tc.tile_pool(name="pt", bufs=6) as ptp,
            tc.tile_pool(name="osb", bufs=2) as osbp,
            tc.tile_pool(name="outsb", bufs=2) as otp,
            tc.tile_pool(name="rc", bufs=4) as rcp,
            tc.tile_pool(name="psS", bufs=2, space="PSUM") as psS,
            tc.tile_pool(name="psK", bufs=2, space="PSUM") as psK,
            tc.tile_pool(name="psO", bufs=2, space="PSUM") as psO,
        ):
            wkv = const.tile([128, NCHI, 128], BF16, tag="wkv")
            mk = const.tile([128, 128], BF16, tag="mk")
            wm = const.tile([128, 16], F32, tag="wm")
            eyb = const.tile([64, 64], F32, tag="eyb")
            eyf = const.tile([65, 65], F32, tag="eyf")
            kt = const.tile([64, T], BF16, tag="kt")
            vaug = const.tile([128, 4 * NPAN, 65], BF16, tag="vaug")  # V|1

            nc.sync.dma_start(wkv[:, 0:2], wkv_d[:, 0:2])
            nc.sync.dma_start(wkv[:, 2:NCHI], wkv_d[:, 2:NCHI])

            xts = {}

            def dma_panel(p, fine=False):
                subs = []
                per = 1 if fine else 2
                for q in range(NCHI // per):
                    xt = xtp.tile([128, per, PAN], BF16, tag=f"xt{per}",
                                  name=f"xt{p}_{q}")
                    eng = nc.sync if q % 2 == 0 else nc.gpsimd
                    eng.dma_start(xt[:], xt_d[:, p, per * q:per * q + per])
                    subs.append(xt)
                xts[p] = (subs, per)

            kvs = {}

            def proj(p, ci):
                if ci == 0:
                    kvs[p] = psK.tile([128, PAN], F32, tag="kv", name=f"kv{p}")
                subs, per = xts[p]
                nc.tensor.matmul(
                    kvs[p][:], wkv[:, ci, :], subs[ci // per][:, ci % per, :],
                    start=(ci == 0), stop=(ci == NCHI - 1),
                )
                if ci == NCHI - 1:
                    kv = kvs[p]
                    nc.vector.tensor_copy(kt[:, p * PAN:(p + 1) * PAN], kv[0:64, :])
                    vh = vhp.tile([64, PAN], F32, tag="vh")
                    nc.vector.tensor_copy(vh[:], kv[64:128, :])
                    kvs[p] = vh

            def vtrans(p):
                """Transpose V panel and append the ones column (unmasked)."""
                vh = kvs.pop(p)
                kv2 = psK.tile([128, PAN], F32, tag="kv")
                for tb in range(4):
                    nc.tensor.transpose(
                        kv2[:, tb * 64:(tb + 1) * 64],
                        vh[:, tb * 128:(tb + 1) * 128], eyb,
                    )
                vv = kv2[:, 0:256].rearrange("p (a b) -> p a b", a=4)
                nc.vector.tensor_copy(vaug[:, 4 * p:4 * p + 4, 0:64], vv)
                nc.vector.memset(vaug[:, 4 * p:4 * p + 4, 64:65], 1.0)

            acc = [None]

            def pv(kb, rhs, region, first=False, stop=False):
                nc.tensor.matmul(
                    acc[0][:, region[0]:region[1]], vaug[:, kb, :], rhs,
                    start=first, stop=stop,
                )

            def off_pair(ph, kb_a, kb_b, first=False):
                """Two full key blocks x 512 queries sharing one exp."""
                Q = kt[:, ph * GRP:(ph + 1) * GRP]
                s = psS.tile([128, 1024], F32, tag="ps")
                for j, kb in enumerate((kb_a, kb_b)):
                    nc.tensor.matmul(
                        s[:, 512 * j:512 * j + 512],
                        kt[:, kb * 128:(kb + 1) * 128], Q[:],
                        start=True, stop=True,
                    )
                pt = ptp.tile([128, 1024], BF16, tag="pt")
                nc.scalar.activation(pt[:], s[:], EXP, scale=0.125)
                if kb_a in WM[ph]:
                    wcol = wm[:, 4 * ph + WM[ph].index(kb_a):
                              4 * ph + WM[ph].index(kb_a) + 1]
                    nc.vector.tensor_scalar_mul(pt[:], pt[:], wcol)
                if first:
                    acc[0] = psO.tile([65, GRP], F32, tag="ot", name=f"ot{ph}")
                pv(kb_a, pt[:, 0:512], (0, 512), first=first)
                pv(kb_b, pt[:, 512:1024], (0, 512))

            def ladder(ph, which, stop=False):
                """Diagonal ladder pair: (c0=0,128) or (c0=256,384)."""
                Q = kt[:, ph * GRP:(ph + 1) * GRP]
                kb0 = 4 * ph + 2 * which
                if which == 0:
                    segs = [(kb0, 0, 0, 512), (kb0 + 1, 128, 512, 896)]
                    width = 896
                else:
                    segs = [(kb0, 256, 0, 256), (kb0 + 1, 384, 256, 384)]
                    width = 384
                s = psS.tile([128, 1024], F32, tag="ps")
                for kb, c0, o, e in segs:
                    nc.tensor.matmul(
                        s[:, o:e], kt[:, kb * 128:(kb + 1) * 128],
                        Q[:, c0:GRP], start=True, stop=True,
                    )
                pt = ptp.tile([128, 1024], BF16, tag="pt")
                nc.scalar.activation(pt[:, 0:width], s[:, 0:width], EXP, scale=0.125)
                for kb, c0, o, e in segs:
                    nc.vector.tensor_mul(pt[:, o:o + 128], pt[:, o:o + 128], mk[:])
                for kb, c0, o, e in segs:
                    pv(kb, pt[:, o:e], (c0, GRP), stop=stop)

            def ep_copy(ph):
                osb = osbp.tile([65, GRP], F32, tag="osb")
                nc.vector.tensor_copy(osb[:], acc[0][:])
                return osb

            def epilogue(ph, osb, pool):
                outsb = otp.tile([128, 4, H], F32, tag="outsb")
                for k in range(2):
                    tl = pool.tile([128, pool_w[id(pool)]], F32, tag=pool_tag[id(pool)])
                    for j in range(2):
                        i = 2 * k + j
                        nc.tensor.transpose(
                            tl[:, 256 * j:256 * j + 65],
                            osb[:, i * 128:(i + 1) * 128], eyf,
                        )
                    for j in range(2):
                        i = 2 * k + j
                        rc = rcp.tile([128, 1], F32, tag="rc")
                        nc.vector.reciprocal(rc[:], tl[:, 256 * j + 64:256 * j + 65])
                        nc.vector.tensor_scalar_mul(
                            outsb[:, i, :], tl[:, 256 * j:256 * j + 64], rc[:]
                        )
                nc.sync.dma_start(out_v[:, 4 * ph:4 * ph + 4, :], outsb[:])

            pool_w = {id(psS): 1024, id(psK): PAN}
            pool_tag = {id(psS): "ps", id(psK): "kv"}

            def warmup(n, pool, w, tag):
                """Dependency-free PE work: holds the DVFS p-state at full
                clock through head-region DMA stalls (output never read)."""
                for k in range(n):
                    wu = pool.tile([128, w], F32, tag=tag, name=f"wu{tag}{k}")
                    nc.tensor.matmul(
                        wu[:, 0:128], wkv[:, 0, :], wkv[:, 1, :],
                        start=True, stop=True,
                    )

            # ---- schedule ----
            dma_panel(3, fine=True)
            dma_panel(4, fine=True)
            warmup(12, psK, PAN, "kv")
            nc.gpsimd.dma_start(eyb[:], eyb_d[:])
            nc.gpsimd.dma_start(mk[:], mk_d[:])
            nc.gpsimd.dma_start(wm[:], wm_d[:])
            nc.gpsimd.dma_start(eyf[:], eyf_d[:])
            for ci in range(NCHI):
                proj(3, ci)
            for ci in range(NCHI):
                proj(4, ci)
            vtrans(3)
            vtrans(4)
            warmup(12, psS, 1024, "ps")

            fill = []
            for p in (2, 6, 1, 7, 0, 5):
                fill += [(p, u) for u in range(9)]
            fidx = [0]

            def filler(n):
                for _ in range(n):
                    if fidx[0] >= len(fill):
                        return
                    p, u = fill[fidx[0]]
                    fidx[0] += 1
                    if u == 8:
                        vtrans(p)
                    else:
                        proj(p, u)

            pace = {**{i: 3 for i in range(9)}, **{i: 2 for i in range(9, 16)},
                    **{i: 1 for i in range(16, 23)},
                    **{i: 1 for i in range(28, 34)}}
            dma_at = {0: 2, 1: 6, 4: 1, 6: 7, 9: 0, 12: 5}

            pair_idx = 0
            pending_ep = []

            def tick():
                nonlocal pair_idx
                if pair_idx in dma_at:
                    dma_panel(dma_at[pair_idx])
                filler(pace.get(pair_idx, 0))
                pair_idx += 1

            for n_ph, ph in enumerate(PH_ORDER):
                offs = LOFF[ph]
                for x in range(0, len(offs), 2):
                    off_pair(ph, offs[x], offs[x + 1], first=(x == 0))
                    tick()
                ladder(ph, 0)
                tick()
                ladder(ph, 1, stop=True)
                tick()
                osb = ep_copy(ph)
                pending_ep.append((ph, osb))
                if n_ph == len(PH_ORDER) - 1:
                    filler(len(fill))
                    for ph2, osb2 in pending_ep[:-1]:
                        epilogue(ph2, osb2, psK)
                    epilogue(ph, osb, psS)
                elif len(pending_ep) > 2:
                    ph2, osb2 = pending_ep.pop(0)
                    epilogue(ph2, osb2, psK)

    nc.compile()
    return nc


def make_inputs(x, Wk, Wv):
    bf16 = ml_dtypes.bfloat16
    wkv = np.concatenate([Wk, Wv], axis=1)
    wkv_t = wkv.reshape(NCHI, 128, 128).transpose(1, 0, 2).astype(bf16)

    pp = np.arange(128)[:, None]
    jj = np.arange(128)[None, :]
    mk = (jj >= pp).astype(bf16)
    eyb = np.eye(64, dtype=np.float32)
    eyf = np.eye(65, dtype=np.float32)

    in_maps = []
    for c in range(8):
        b, role = divmod(c, 2)
        pan = GA[role]
        xT = np.ascontiguousarray(x[b].T)
        xr = xT.reshape(NCHI, 128, T)
        xt = np.empty((128, NPAN, NCHI, PAN), dtype=bf16)
        for j, pg in enumerate(pan):
            xt[:, j] = xr[:, :, pg * PAN:(pg + 1) * PAN].transpose(1, 0, 2)

        wmv = np.zeros((128, 16), dtype=np.float32)
        for ph in range(4):
            wmv[:, 4 * ph:4 * ph + 4] = WMVAL[role][ph]

        in_maps.append(
            {"xt": xt, "wkv": wkv_t, "mk": mk, "wm": wmv, "eyb": eyb,
             "eyf": eyf}
        )
    return in_maps


_NC = None


def get_nc():
    global _NC
    if _NC is None:
        _NC = build_nc()
    return _NC


def kernel(x, Wk, Wv):
    x = np.asarray(x, dtype=np.float32)
    Wk = np.asarray(Wk, dtype=np.float32)
    Wv = np.asarray(Wv, dtype=np.float32)
    nc = get_nc()
    in_maps = make_inputs(x, Wk, Wv)
    res = run_bass_kernel_spmd(nc, in_maps, list(range(8)))
    out = np.empty((B, T, H), dtype=np.float32)
    for c in range(8):
        b, role = divmod(c, 2)
        o = res.results[c]["out"]
        for ph in range(4):
            g = GA[role][ph]
            out[b, g * GRP:(g + 1) * GRP] = o[ph * GRP:(ph + 1) * GRP]
    return out


# revision 30
# speedup vs baseline: 1.0320x; 1.0320x over previous
"""Causal single-head attention (Q==K) on 8 TRN2 NeuronCores — v3.

v3 restructure: queries split into 512-row groups (8 per batch); each
core of a batch pair owns 4 groups chosen so causal work balances with
near-zero overcompute (8 of 512 key-block-slots wasted per core vs 16
in v2).  Four sequential accumulation phases (one PSUM bank each,
double buffered), processed lightest-first so the panel DMA/projection
pipeline ramps.  Key-block validity for possibly-wasted fixed stream
slots is per-core data (wmask column multiply on the exp'd weights).
"""
import numpy as np
import ml_dtypes

import concourse.bass as bass
import concourse.mybir as mybir
from concourse import bacc, tile
from concourse.bass_utils import run_bass_kernel_spmd

F32 = mybir.dt.float32
BF16 = mybir.dt.bfloat16
EXP = mybir.ActivationFunctionType.Exp

B, T, C, H = 4, 4096, 1024, 64
NCHI = C // 128
NPAN = 8
PAN = 512
GRP = 512                      # query group rows
# structural phase i: query cols [512i, 512i+512), diag kbs 4i..4i+3
LOFF = {
    0: list(range(4, 20)) + list(range(24, 32)) + list(range(20, 24)),
    1: list(range(8, 20)) + list(range(24, 32)),
    2: list(range(12, 20)) + list(range(24, 28)),
    3: list(range(16, 20)),
}
WM = {0: (20, 21, 22, 23), 1: (28, 29, 30, 31),
      2: (24, 25, 26, 27), 3: (16, 17, 18, 19)}
PH_ORDER = [3, 2, 1, 0]        # execution order: lightest first
# role -> structural phase i holds this global 512-group; panels 4..7 hold
# the remaining groups in this order
GA = {0: [7, 4, 3, 0, 1, 5, 2, 6], 1: [6, 5, 2, 1, 0, 7, 3, 4]}
# role -> wmask value per phase (1 = the WM slots are valid keys)
WMVAL = {0: (1.0, 0.0, 1.0, 0.0), 1: (0.0, 1.0, 0.0, 1.0)}


def build_nc():
    nc = bacc.Bacc("TRN2", target_bir_lowering=False, debug=False, num_devices=8)

    xt_d = nc.declare_dram_parameter("xt", [128, NPAN, NCHI, PAN], BF16, isOutput=False)
    wkv_d = nc.declare_dram_parameter("wkv", [128, NCHI, 128], BF16, isOutput=False)
    mk_d = nc.declare_dram_parameter("mk", [128, 128], BF16, isOutput=False)
    wm_d = nc.declare_dram_parameter("wm", [128, 16], F32, isOutput=False)
    eyb_d = nc.declare_dram_parameter("eyb", [64, 64], F32, isOutput=False)
    eyf_d = nc.declare_dram_parameter("eyf", [65, 65], F32, isOutput=False)
    out_d = nc.declare_dram_parameter("out", [4 * GRP, H], F32, isOutput=True)
    out_v = out_d.ap().rearrange("(i p) h -> p i h", p=128)  # [128, 16, 64]

    with tile.TileContext(nc) as tc:
        with (
            tc.tile_pool(name="const", bufs=1) as const,
            tc.tile_pool(name="xt", bufs=12) as xtp,
            tc.tile_pool(name="vh", bufs=2) as vhp,
            tc.tile_pool(name="pt", bufs=6) as ptp,
            tc.tile_pool(name="osb", bufs=2) as osbp,
            tc.tile_pool(name="outsb", bufs=2) as otp,
            tc.tile_pool(name="rc", bufs=4) as rcp,
            tc.tile_pool(name="psS", bufs=2, space="PSUM") as psS,
            tc.tile_pool(name="psK", bufs=2, space="PSUM") as psK,
            tc.tile_pool(name="psO", bufs=2, space="PSUM") as psO,
        ):
            wkv = const.tile([128, NCHI, 128], BF16, tag="wkv")
            mk = const.tile([128, 128], BF16, tag="mk")
            wm = const.tile([128, 16], F32, tag="wm")
            eyb = const.tile([64, 64], F32, tag="eyb")
            eyf = const.tile([65, 65], F32, tag="eyf")
            kt = const.tile([64, T], BF16, tag="kt")
            vaug = const.tile([128, 4 * NPAN, 65], BF16, tag="vaug")  # V|1

            nc.sync.dma_start(wkv[:, 0:2], wkv_d[:, 0:2])
            nc.sync.dma_start(wkv[:, 2:NCHI], wkv_d[:, 2:NCHI])

            xts = {}

            def dma_panel(p, fine=False):
                subs = []
                per = 1 if fine else 2
                for q in range(NCHI // per):
                    xt = xtp.tile([128, per, PAN], BF16, tag=f"xt{per}",
                                  name=f"xt{p}_{q}")
                    eng = nc.sync if q % 2 == 0 else nc.gpsimd
                    eng.dma_start(xt[:], xt_d[:, p, per * q:per * q + per])
                    subs.append(xt)
                xts[p] = (subs, per)

            kvs = {}

            def proj(p, ci):
                if ci == 0:
                    kvs[p] = psK.tile([128, PAN], F32, tag="kv", name=f"kv{p}")
                subs, per = xts[p]
                nc.tensor.matmul(
                    kvs[p][:], wkv[:, ci, :], subs[ci // per][:, ci % per, :],
                    start=(ci == 0), stop=(ci == NCHI - 1),
                )
                if ci == NCHI - 1:
                    kv = kvs[p]
                    nc.vector.tensor_copy(kt[:, p * PAN:(p + 1) * PAN], kv[0:64, :])
                    vh = vhp.tile([64, PAN], F32, tag="vh")
                    nc.vector.tensor_copy(vh[:], kv[64:128, :])
                    kvs[p] = vh

            def vtrans(p):
                """Transpose V panel and append the ones column (unmasked)."""
                vh = kvs.pop(p)
                kv2 = psK.tile([128, PAN], F32, tag="kv")
                for tb in range(4):
                    nc.tensor.transpose(
                        kv2[:, tb * 64:(tb + 1) * 64],
                        vh[:, tb * 128:(tb + 1) * 128], eyb,
                    )
                vv = kv2[:, 0:256].rearrange("p (a b) -> p a b", a=4)
                nc.vector.tensor_copy(vaug[:, 4 * p:4 * p + 4, 0:64], vv)
                nc.vector.memset(vaug[:, 4 * p:4 * p + 4, 64:65], 1.0)

            acc = [None]

            def pv(kb, rhs, region, first=False, stop=False):
                nc.tensor.matmul(
                    acc[0][:, region[0]:region[1]], vaug[:, kb, :], rhs,
                    start=first, stop=stop,
                )

            def off_pair(ph, kb_a, kb_b, first=False):
                """Two full key blocks x 512 queries sharing one exp."""
                Q = kt[:, ph * GRP:(ph + 1) * GRP]
                s = psS.tile([128, 1024], F32, tag="ps")
                for j, kb in enumerate((kb_a, kb_b)):
                    nc.tensor.matmul(
                        s[:, 512 * j:512 * j + 512],
                        kt[:, kb * 128:(kb + 1) * 128], Q[:],
                        start=True, stop=True,
                    )
                pt = ptp.tile([128, 1024], BF16, tag="pt")
                nc.scalar.activation(pt[:], s[:], EXP, scale=0.125)
                if kb_a in WM[ph]:
                    wcol = wm[:, 4 * ph + WM[ph].index(kb_a):
                              4 * ph + WM[ph].index(kb_a) + 1]
                    nc.vector.tensor_scalar_mul(pt[:], pt[:], wcol)
                if first:
                    acc[0] = psO.tile([65, GRP], F32, tag="ot", name=f"ot{ph}")
                pv(kb_a, pt[:, 0:512], (0, 512), first=first)
                pv(kb_b, pt[:, 512:1024], (0, 512))

            def ladder(ph, which, stop=False):
                """Diagonal ladder pair: (c0=0,128) or (c0=256,384)."""
                Q = kt[:, ph * GRP:(ph + 1) * GRP]
                kb0 = 4 * ph + 2 * which
                if which == 0:
                    segs = [(kb0, 0, 0, 512), (kb0 + 1, 128, 512, 896)]
                    width = 896
                else:
                    segs = [(kb0, 256, 0, 256), (kb0 + 1, 384, 256, 384)]
                    width = 384
                s = psS.tile([128, 1024], F32, tag="ps")
                for kb, c0, o, e in segs:
                    nc.tensor.matmul(
                        s[:, o:e], kt[:, kb * 128:(kb + 1) * 128],
                        Q[:, c0:GRP], start=True, stop=True,
                    )
                pt = ptp.tile([128, 1024], BF16, tag="pt")
                nc.scalar.activation(pt[:, 0:width], s[:, 0:width], EXP, scale=0.125)
                for kb, c0, o, e in segs:
                    nc.vector.tensor_mul(pt[:, o:o + 128], pt[:, o:o + 128], mk[:])
                for kb, c0, o, e in segs:
                    pv(kb, pt[:, o:e], (c0, GRP), stop=stop)

            def ep_copy(ph):
                osb = osbp.tile([65, GRP], F32, tag="osb")
                nc.vector.tensor_copy(osb[:], acc[0][:])
                return osb

            def epilogue(ph, osb, pool):
                outsb = otp.tile([128, 4, H], F32, tag="outsb")
                for k in range(2):
                    tl = pool.tile([128, pool_w[id(pool)]], F32, tag=pool_tag[id(pool)])
                    for j in range(2):
                        i = 2 * k + j
                        nc.tensor.transpose(
                            tl[:, 256 * j:256 * j + 65],
                            osb[:, i * 128:(i + 1) * 128], eyf,
                        )
                    for j in range(2):
                        i = 2 * k + j
                        rc = rcp.tile([128, 1], F32, tag="rc")
                        nc.vector.reciprocal(rc[:], tl[:, 256 * j + 64:256 * j + 65])
                        nc.vector.tensor_scalar_mul(
                            outsb[:, i, :], tl[:, 256 * j:256 * j + 64], rc[:]
                        )
                nc.sync.dma_start(out_v[:, 4 * ph:4 * ph + 4, :], outsb[:])

            pool_w = {id(psS): 1024, id(psK): PAN}
            pool_tag = {id(psS): "ps", id(psK): "kv"}

            def warmup(n, pool, w, tag):
                """Dependency-free PE work: holds the DVFS p-state at full
                clock through head-region DMA stalls (output never read)."""
                for k in range(n):
                    wu = pool.tile([128, w], F32, tag=tag, name=f"wu{tag}{k}")
                    nc.tensor.matmul(
                        wu[:, 0:128], wkv[:, 0, :], wkv[:, 1, :],
                        start=True, stop=True,
                    )

            # ---- schedule ----
            dma_panel(3, fine=True)
            dma_panel(4, fine=True)
            warmup(6, psK, PAN, "kv")
            nc.gpsimd.dma_start(eyb[:], eyb_d[:])
            nc.gpsimd.dma_start(mk[:], mk_d[:])
            nc.gpsimd.dma_start(wm[:], wm_d[:])
            nc.gpsimd.dma_start(eyf[:], eyf_d[:])
            for ci in range(NCHI):
                proj(3, ci)
            warmup(5, psS, 1024, "ps")
            for ci in range(NCHI):
                proj(4, ci)
            warmup(5, psS, 1024, "ps")
            vtrans(3)
            vtrans(4)
            warmup(5, psS, 1024, "ps")

            fill = []
            for p in (2, 6, 1, 7, 0, 5):
                fill += [(p, u) for u in range(9)]
            fidx = [0]

            def filler(n):
                for _ in range(n):
                    if fidx[0] >= len(fill):
                        return
                    p, u = fill[fidx[0]]
                    fidx[0] += 1
                    if u == 8:
                        vtrans(p)
                    else:
                        proj(p, u)

            pace = {**{i: 3 for i in range(9)}, **{i: 2 for i in range(9, 16)},
                    **{i: 1 for i in range(16, 23)},
                    **{i: 1 for i in range(28, 34)}}
            dma_at = {0: 2, 1: 6, 4: 1, 6: 7, 9: 0, 12: 5}

            pair_idx = 0
            pending_ep = []

            def tick():
                nonlocal pair_idx
                if pair_idx in dma_at:
                    dma_panel(dma_at[pair_idx])
                filler(pace.get(pair_idx, 0))
                pair_idx += 1

            for n_ph, ph in enumerate(PH_ORDER):
                offs = LOFF[ph]
                for x in range(0, len(offs), 2):
                    off_pair(ph, offs[x], offs[x + 1], first=(x == 0))
                    tick()
                ladder(ph, 0)
                tick()
                ladder(ph, 1, stop=True)
                tick()
                osb = ep_copy(ph)
                pending_ep.append((ph, osb))
                if n_ph == len(PH_ORDER) - 1:
                    filler(len(fill))
                    for ph2, osb2 in pending_ep[:-1]:
                        epilogue(ph2, osb2, psK)
                    epilogue(ph, osb, psS)
                elif len(pending_ep) > 2:
                    ph2, osb2 = pending_ep.pop(0)
                    epilogue(ph2, osb2, psK)

    nc.compile()
    return nc


def make_inputs(x, Wk, Wv):
    bf16 = ml_dtypes.bfloat16
    wkv = np.concatenate([Wk, Wv], axis=1)
    wkv_t = wkv.reshape(NCHI, 128, 128).transpose(1, 0, 2).astype(bf16)

    pp = np.arange(128)[:, None]
    jj = np.arange(128)[None, :]
    mk = (jj >= pp).astype(bf16)
    eyb = np.eye(64, dtype=np.float32)
    eyf = np.eye(65, dtype=np.float32)

    in_maps = []
    for c in range(8):
        b, role = divmod(c, 2)
        pan = GA[role]
        xT = np.ascontiguousarray(x[b].T)
        xr = xT.reshape(NCHI, 128, T)
        xt = np.empty((128, NPAN, NCHI, PAN), dtype=bf16)
        for j, pg in enumerate(pan):
            xt[:, j] = xr[:, :, pg * PAN:(pg + 1) * PAN].transpose(1, 0, 2)

        wmv = np.zeros((128, 16), dtype=np.float32)
        for ph in range(4):
            wmv[:, 4 * ph:4 * ph + 4] = WMVAL[role][ph]

        in_maps.append(
            {"xt": xt, "wkv": wkv_t, "mk": mk, "wm": wmv, "eyb": eyb,
             "eyf": eyf}
        )
    return in_maps


_NC = None


def get_nc():
    global _NC
    if _NC is None:
        _NC = build_nc()
    return _NC


def kernel(x, Wk, Wv):
    x = np.asarray(x, dtype=np.float32)
    Wk = np.asarray(Wk, dtype=np.float32)
    Wv = np.asarray(Wv, dtype=np.float32)
    nc = get_nc()
    in_maps = make_inputs(x, Wk, Wv)
    res = run_bass_kernel_spmd(nc, in_maps, list(range(8)))
    out = np.empty((B, T, H), dtype=np.float32)
    for c in range(8):
        b, role = divmod(c, 2)
        o = res.results[c]["out"]
        for ph in range(4):
            g = GA[role][ph]
            out[b, g * GRP:(g + 1) * GRP] = o[ph * GRP:(ph + 1) * GRP]
    return out
